# revision 3
# baseline (speedup 1.0000x reference)
"""Conv1D-FFT-autograd forward on 8 NeuronCores.

Pipeline (per sample, exact reference semantics):
  rfft(x, 131072) -> keep bins 0..32768 -> einsum over C with conj(yfft)
  -> zero-pad high bins -> irfft(131072) -> first 65409 samples + bias.

The 131072-point FFTs are factored into two matmul stages (radix 512x256)
with an elementwise twiddle between them, so everything lowers to real
matmuls + elementwise ops the NeuronCore compiles natively.  Data-parallel
over N: each of the 8 cores runs 4 samples; the small filter spectrum is
replicated as a compile-time constant.

Index math (w = 512*m + a, k = r + 256*s, t = a2 + 512*p, N = 131072):
  fwd:  X[r+256s] = sum_a F[a,s] * twid[a,r] * sum_m E[m,r] * x[512m+a]
        E[m,r] = exp(-2i*pi*m*r/256),  twid = exp(-2i*pi*a*r/N),
        F[a,s] = exp(-2i*pi*a*s/512)
  inv:  v[a2+512p] = sum_r E2[r,p] * twid2[r,a2] * sum_s F2[s,a2] * Z[r+256s]
        out = (2/N) * Re(v)   (bin 0 pre-halved; bins>32768 zero)
"""

import numpy as np

N_FULL, C, W = 32, 8, 65536
F, WW = 16, 128
NFFT = 131072
KEEP = 32769           # bins 0..32768
OUT_W = W - WW + 1     # 65409
NDEV = 8
NS = N_FULL // NDEV    # 4 samples per core

_jitted = None


def _build():
    import jax
    import jax.numpy as jnp
    from functools import partial

    m = np.arange(128.0)
    r = np.arange(256.0)
    a = np.arange(512.0)
    s = np.arange(129.0)
    p = np.arange(128.0)

    def cs(theta):
        return (np.cos(theta).astype(np.float32), np.sin(theta).astype(np.float32))

    # forward constants
    Er, Ei = cs(-2 * np.pi * np.outer(m, r) / 256.0)          # [128,256]
    Twr, Twi = cs(-2 * np.pi * np.outer(a, r) / NFFT)         # [512,256]
    Fr, Fi = cs(-2 * np.pi * np.outer(a, s) / 512.0)          # [512,129]
    # inverse constants
    F2r, F2i = cs(+2 * np.pi * np.outer(s, a) / 512.0)        # [129,512]
    W2r, W2i = cs(+2 * np.pi * np.outer(r, a) / NFFT)         # [256,512]
    E2r, E2i = cs(+2 * np.pi * np.outer(r, p) / 256.0)        # [256,128]

    # mask: grid position (s,r) is a kept bin iff k = r+256s <= 32768
    mask = np.ones((129, 256), np.float32)
    mask[128, 1:] = 0.0
    # DC bin halved so the inverse is (2/N) * Re(sum over kept bins)
    halfmask = np.ones((129, 256), np.float32)
    halfmask[0, 0] = 0.5

    def shard_fn(Yr, Yi, bias, xs):
        # xs: [NS, C, W] float32
        z = xs.reshape(NS, C, 128, 512)                       # z[n,c,m,a]
        Gr = jnp.einsum("ncma,mr->ncar", z, Er)
        Gi = jnp.einsum("ncma,mr->ncar", z, Ei)
        Tr = Gr * Twr - Gi * Twi                              # [n,c,a,r]
        Ti = Gr * Twi + Gi * Twr
        Xr = jnp.einsum("ncar,as->ncsr", Tr, Fr) - jnp.einsum("ncar,as->ncsr", Ti, Fi)
        Xi = jnp.einsum("ncar,as->ncsr", Tr, Fi) + jnp.einsum("ncar,as->ncsr", Ti, Fr)
        Xr = Xr * mask
        Xi = Xi * mask
        # spectral cross-correlation, summed over C:  Z = X * conj(Y)
        Zr = jnp.einsum("ncsr,fcsr->nfsr", Xr, Yr) + jnp.einsum("ncsr,fcsr->nfsr", Xi, Yi)
        Zi = jnp.einsum("ncsr,fcsr->nfsr", Xi, Yr) - jnp.einsum("ncsr,fcsr->nfsr", Xr, Yi)
        # halve DC bin so out = (2/N) Re(sum over kept bins)
        Zr = Zr * halfmask
        Zi = Zi * halfmask
        Ur = jnp.einsum("nfsr,sa->nfra", Zr, F2r) - jnp.einsum("nfsr,sa->nfra", Zi, F2i)
        Ui = jnp.einsum("nfsr,sa->nfra", Zr, F2i) + jnp.einsum("nfsr,sa->nfra", Zi, F2r)
        U2r = Ur * W2r - Ui * W2i                             # [n,f,r,a2]
        U2i = Ur * W2i + Ui * W2r
        vr = jnp.einsum("nfra,rp->nfpa", U2r, E2r) - jnp.einsum("nfra,rp->nfpa", U2i, E2i)
        out = (2.0 / NFFT) * vr.reshape(NS, F, W)[:, :, :OUT_W]
        return out + bias[None, :, None]

    return jax.jit(shard_fn)


def kernel(x, weight, bias):
    global _jitted
    import jax

    x = np.ascontiguousarray(x, np.float32)
    weight = np.ascontiguousarray(weight, np.float32)
    bias = np.ascontiguousarray(bias, np.float32)

    # replicated filter spectrum, computed host-side (F*C*KEEP ~ 4M bins)
    Y = np.fft.rfft(weight.astype(np.float64), n=NFFT, axis=-1)[..., :KEEP]
    k = np.arange(KEEP)
    Ysr = np.zeros((F, C, 129, 256), np.complex128)
    Ysr[:, :, k // 256, k % 256] = Y
    Yr = np.ascontiguousarray(Ysr.real, np.float32)
    Yi = np.ascontiguousarray(Ysr.imag, np.float32)

    if _jitted is None:
        _jitted = _build()

    devs = jax.devices()[:NDEV]
    futs = []
    for i, d in enumerate(devs):
        xs = jax.device_put(x[i * NS:(i + 1) * NS], d)
        yr = jax.device_put(Yr, d)
        yi = jax.device_put(Yi, d)
        b = jax.device_put(bias, d)
        futs.append(_jitted(yr, yi, b, xs))
    out = np.concatenate([np.asarray(f) for f in futs], axis=0)
    return out.astype(np.float32)
